# revision 5
# baseline (speedup 1.0000x reference)
"""Bass/Trainium2 kernel for BiasedAttention (B=8, N=2048, H=256), SPMD over 8 cores.

Per-core work (data-parallel over batch): one batch element.
  Q = x@Wq*s + bq*s ; K = x@Wk + bk ; V = x@Wv + bv
  S = Q K^T + attn_bias ; P = exp(S) (unnormalized, values are small enough)
  O = (P @ [V|1]) -> numerator and denominator in one matmul ; out = (O/den) @ Wo + bo

Layout strategy: keep Q^T,K^T [H,N] on-chip (h on partitions); scores computed
[q,k] (bias streams naturally); P transposed via PE identity-matmuls to feed
the P@V contraction; O transposed the same way for the final projection.
"""

import sys

for _p in ("/opt/trn_rl_repo", "/root/.axon_site/_ro/trn_rl_repo"):
    if _p not in sys.path:
        sys.path.append(_p)

import numpy as np

import concourse.bass as bass
import concourse.tile as tile
from concourse import mybir
from concourse.bass_utils import run_bass_kernel_spmd
from concourse.vector_clock import ScopedClock

B, N, H = 8, 2048, 256
SCALE = H ** -0.5
P = 128
NT = N // P          # 16 row tiles per core
HC = H // P          # 2 h chunks
KC = N // 512        # 4 k chunks of 512
F32 = mybir.dt.float32
BF16 = mybir.dt.bfloat16


def _patch_tile_drain():
    """walrus here rejects >1 sync-wait on a CTRL/Drain instruction; split the
    TileContext exit-drain's waits across a chain of drains."""
    if getattr(tile.TileContext, "_drain_patched", False):
        return

    def _drain_and_barrier(self, tick_clock, wait_clock):
        drain_inst = self.nc.sync.drain()
        wait_clock.add_sem_waits(
            drain_inst.ins, ScopedClock({None: tick_clock.global_clock})
        )
        si = drain_inst.ins.sync_info
        waits = list(si.on_wait) if si is not None and si.on_wait else []
        if len(waits) > 1:
            drain_inst.ins.sync_info = mybir.SyncInfo(on_wait=waits[:1], on_update=[])
            for w in waits[1:]:
                d2 = self.nc.sync.drain()
                d2.ins.sync_info = mybir.SyncInfo(on_wait=[w], on_update=[])
        self.nc.all_engine_barrier()
        assert self.sems is not None
        popped = self.nc._tile_sem_poison_stack.pop()
        assert popped is self._sem_poison
        self.nc.clear_and_free_semaphores(list(self.sems.allocated().values()))
        self.nc.all_engine_barrier()

    tile.TileContext._drain_and_barrier = _drain_and_barrier
    tile.TileContext._drain_patched = True


MAX_SYNC_WAITS = 1


def _split_sync_waits(nc: bass.Bass, max_waits: int = MAX_SYNC_WAITS):
    """walrus rejects instructions with too many sync waits; hoist the excess
    onto same-engine NOPs inserted just before the instruction."""
    for fn in nc.m.functions:
        for bb in fn.blocks:
            new = []
            for inst in bb.instructions:
                si = inst.sync_info
                waits = list(si.on_wait) if si is not None and si.on_wait else []
                if len(waits) > max_waits:
                    inst.sync_info = mybir.SyncInfo(
                        on_wait=waits[-max_waits:],
                        on_update=list(si.on_update) if si.on_update else [],
                    )
                    excess = waits[:-max_waits]
                    for i in range(0, len(excess), max_waits):
                        nop = mybir.InstNoOp(
                            name=nc.get_next_instruction_name(),
                            sync_info=mybir.SyncInfo(
                                on_wait=excess[i:i + max_waits], on_update=[]
                            ),
                            engine=inst.engine,
                            bass_nofuse=True,
                        )
                        new.append(nop)
                new.append(inst)
            bb.instructions[:] = new


def build_program() -> bass.Bass:
    _patch_tile_drain()
    nc = bass.Bass()
    Exp = mybir.ActivationFunctionType.Exp

    x_d = nc.declare_dram_parameter("x", [N, H], F32, isOutput=False)
    ab_d = nc.declare_dram_parameter("ab", [N, N], F32, isOutput=False)
    w_d = {
        k: nc.declare_dram_parameter(k, [H, H], F32, isOutput=False)
        for k in ("wq", "wk", "wv", "wo")
    }
    b_d = {
        k: nc.declare_dram_parameter(k, [1, H], F32, isOutput=False)
        for k in ("bq", "bk", "bv", "bo")
    }
    id_d = nc.declare_dram_parameter("ident", [P, P], F32, isOutput=False)
    y_d = nc.declare_dram_parameter("y", [N, H], F32, isOutput=True)

    with tile.TileContext(nc) as tc:
        with (
            tc.tile_pool(name="const", bufs=1) as const,
            tc.tile_pool(name="setup", bufs=2) as setup,
            tc.tile_pool(name="acts", bufs=1) as acts,
            tc.tile_pool(name="bias", bufs=3) as biasp,
            tc.tile_pool(name="s", bufs=2) as sp,
            tc.tile_pool(name="p", bufs=2) as pp,
            tc.tile_pool(name="ptsb", bufs=2) as ptsb,
            tc.tile_pool(name="small", bufs=4) as small,
            tc.tile_pool(name="ysb", bufs=3) as ysb,
            tc.tile_pool(name="mm", bufs=1, space="PSUM") as mmp,
            tc.tile_pool(name="qk", bufs=3, space="PSUM") as qkp,
            tc.tile_pool(name="pt", bufs=2, space="PSUM") as ptp,
            tc.tile_pool(name="o", bufs=2, space="PSUM") as op_,
        ):
            # ---- constants ----
            id_f32 = const.tile([P, P], F32)
            nc.sync.dma_start(out=id_f32[:], in_=id_d[:])
            id_bf = const.tile([P, P], BF16)
            nc.vector.tensor_copy(id_bf[:], id_f32[:])
            ones_r = const.tile([1, 512], BF16)
            nc.vector.memset(ones_r[:], 1.0)

            wsb = {}
            for k in ("wq", "wk", "wv", "wo"):
                wf = setup.tile([P, HC, H], F32, name="wf")
                nc.sync.dma_start(
                    out=wf[:], in_=w_d[k].rearrange("(c p) o -> p c o", p=P)
                )
                wb = const.tile([P, HC, H], BF16, name=f"{k}_bf")
                nc.vector.tensor_copy(wb[:], wf[:])
                wsb[k] = wb
            bsb = {}
            for k in ("bq", "bk", "bv", "bo"):
                bf_ = setup.tile([1, H], F32, name="bf")
                nc.sync.dma_start(out=bf_[:], in_=b_d[k][:])
                bb = const.tile([1, H], BF16, name=f"{k}_bf")
                nc.vector.tensor_copy(bb[:], bf_[:])
                bsb[k] = bb

            # ---- x load + transpose to xT (bf16) ----
            x_sb = setup.tile([P, NT, H], F32, name="x_sb")
            nc.sync.dma_start(out=x_sb[:], in_=x_d.rearrange("(t p) h -> p t h", p=P))
            xt = acts.tile([P, HC, N], BF16, name="xt")
            for hc in range(HC):
                for tg in range(NT // 4):
                    ps = mmp.tile([P, 512], F32, name="mm")
                    for j in range(4):
                        t = tg * 4 + j
                        nc.tensor.matmul(
                            ps[:, j * P:(j + 1) * P],
                            lhsT=x_sb[:, t, hc * P:(hc + 1) * P],
                            rhs=id_f32[:],
                            start=True, stop=True,
                        )
                    nc.vector.tensor_copy(xt[:, hc, tg * 512:(tg + 1) * 512], ps[:])

            # ---- Q^T, K^T (bf16, [h_out part, hc, n]) ----
            qkt = {}
            for name, wkey, bkey in (("qt", "wq", "bq"), ("kt", "wk", "bk")):
                dst = acts.tile([P, HC, N], BF16, name=name)
                for ho in range(HC):
                    for ng in range(KC):
                        ps = mmp.tile([P, 512], F32, name="mm")
                        for hi in range(HC):
                            nc.tensor.matmul(
                                ps[:],
                                lhsT=wsb[wkey][:, hi, ho * P:(ho + 1) * P],
                                rhs=xt[:, hi, ng * 512:(ng + 1) * 512],
                                start=(hi == 0), stop=False,
                            )
                        nc.tensor.matmul(
                            ps[:],
                            lhsT=bsb[bkey][:, ho * P:(ho + 1) * P],
                            rhs=ones_r[:],
                            start=False, stop=True,
                        )
                        nc.scalar.copy(dst[:, ho, ng * 512:(ng + 1) * 512], ps[:])
                qkt[name] = dst
            qt, kt = qkt["qt"], qkt["kt"]

            # ---- V_ext (bf16, [n part, t, h | ones]) ----
            v_sb = acts.tile([P, NT, H + 1], BF16, name="v")
            for t in range(NT):
                ps = mmp.tile([P, 512], F32, name="mm")
                for hi in range(HC):
                    nc.tensor.matmul(
                        ps[:, :H],
                        lhsT=xt[:, hi, t * P:(t + 1) * P],
                        rhs=wsb["wv"][:, hi, :],
                        start=(hi == 0), stop=False,
                    )
                nc.tensor.matmul(
                    ps[:, :H], lhsT=ones_r[:, :P], rhs=bsb["bv"][:],
                    start=False, stop=True,
                )
                nc.scalar.copy(v_sb[:, t, :H], ps[:, :H])
                nc.vector.memset(v_sb[:, t, H:H + 1], 1.0)

            # ---- main loop over q tiles ----
            for qt_i in range(NT):
                b_t = biasp.tile([P, N], F32, name="bias")
                nc.sync.dma_start(out=b_t[:], in_=ab_d[qt_i * P:(qt_i + 1) * P, :])

                s_t = sp.tile([P, N], F32, name="s")
                for kc in range(KC):
                    ps_s = qkp.tile([P, 512], F32, name="qk")
                    for hi in range(HC):
                        nc.tensor.matmul(
                            ps_s[:],
                            lhsT=qt[:, hi, qt_i * P:(qt_i + 1) * P],
                            rhs=kt[:, hi, kc * 512:(kc + 1) * 512],
                            start=(hi == 0), stop=(hi == HC - 1),
                        )
                    nc.vector.tensor_add(
                        s_t[:, kc * 512:(kc + 1) * 512],
                        ps_s[:],
                        b_t[:, kc * 512:(kc + 1) * 512],
                    )

                p_t = pp.tile([P, N], BF16, name="p")
                nc.scalar.activation(p_t[:], s_t[:], Exp)

                pt_t = ptsb.tile([P, NT, P], BF16, name="pt")
                for g in range(4):
                    ps_pt = ptp.tile([P, 512], F32, name="pt_ps")
                    for j in range(4):
                        kc4 = g * 4 + j
                        nc.tensor.matmul(
                            ps_pt[:, j * P:(j + 1) * P],
                            lhsT=p_t[:, kc4 * P:(kc4 + 1) * P],
                            rhs=id_bf[:],
                            start=True, stop=True,
                        )
                    if g % 2 == 0:
                        nc.vector.tensor_copy(pt_t[:, g * 4:(g + 1) * 4, :], ps_pt[:])
                    else:
                        nc.scalar.copy(pt_t[:, g * 4:(g + 1) * 4, :], ps_pt[:])

                ps_o = op_.tile([P, 512], F32, name="o")
                for kc16 in range(NT):
                    nc.tensor.matmul(
                        ps_o[:, :H + 1],
                        lhsT=pt_t[:, kc16, :],
                        rhs=v_sb[:, kc16, :],
                        start=(kc16 == 0), stop=(kc16 == NT - 1),
                    )

                rden = small.tile([P, 1], F32, name="rden")
                nc.vector.reciprocal(rden[:], ps_o[:, H:H + 1])
                o_bf = small.tile([P, H], BF16, name="o_bf")
                nc.vector.tensor_scalar_mul(o_bf[:], ps_o[:, :H], rden[:])

                ps_ot = mmp.tile([P, 512], F32, name="mm")
                for j in range(HC):
                    nc.tensor.matmul(
                        ps_ot[:, j * P:(j + 1) * P],
                        lhsT=o_bf[:, j * P:(j + 1) * P],
                        rhs=id_bf[:],
                        start=True, stop=True,
                    )
                ot_bf = small.tile([P, HC, P], BF16, name="ot_bf")
                nc.vector.tensor_copy(ot_bf[:], ps_ot[:, :H])

                ps_y = mmp.tile([P, 512], F32, name="mm")
                for j in range(HC):
                    nc.tensor.matmul(
                        ps_y[:, :H],
                        lhsT=ot_bf[:, j, :],
                        rhs=wsb["wo"][:, j, :],
                        start=(j == 0), stop=False,
                    )
                nc.tensor.matmul(
                    ps_y[:, :H], lhsT=ones_r[:, :P], rhs=bsb["bo"][:],
                    start=False, stop=True,
                )
                y_t = ysb.tile([P, H], F32, name="y")
                nc.scalar.copy(y_t[:], ps_y[:, :H])
                nc.sync.dma_start(out=y_d[qt_i * P:(qt_i + 1) * P, :], in_=y_t[:])

    _split_sync_waits(nc)
    return nc


_NC = None


def _get_program():
    global _NC
    if _NC is None:
        _NC = build_program()
    return _NC


def make_in_maps(x, attn_bias, W_Q, b_Q, W_K, b_K, W_V, b_V, W_O, b_O):
    f = np.float32
    shared = {
        "wq": np.ascontiguousarray(np.asarray(W_Q, f) * SCALE),
        "wk": np.ascontiguousarray(np.asarray(W_K, f)),
        "wv": np.ascontiguousarray(np.asarray(W_V, f)),
        "wo": np.ascontiguousarray(np.asarray(W_O, f)),
        "bq": np.asarray(b_Q, f).reshape(1, H) * SCALE,
        "bk": np.asarray(b_K, f).reshape(1, H),
        "bv": np.asarray(b_V, f).reshape(1, H),
        "bo": np.asarray(b_O, f).reshape(1, H),
        "ident": np.eye(P, dtype=f),
    }
    x = np.asarray(x, f)
    ab = np.asarray(attn_bias, f)
    return [
        {"x": np.ascontiguousarray(x[b]), "ab": np.ascontiguousarray(ab[b]), **shared}
        for b in range(B)
    ]


def kernel(x, attn_bias, W_Q, b_Q, W_K, b_K, W_V, b_V, W_O, b_O, _trace=False):
    nc = _get_program()
    in_maps = make_in_maps(x, attn_bias, W_Q, b_Q, W_K, b_K, W_V, b_V, W_O, b_O)
    res = run_bass_kernel_spmd(nc, in_maps, core_ids=list(range(B)), trace=_trace)
    out = np.stack([res.results[b]["y"] for b in range(B)], axis=0)
    if _trace:
        kernel.last_results = res
    return out
